# revision 62
# baseline (speedup 1.0000x reference)
"""Trainium2 Bass kernel for nn_AttnResBlockUp (B=16, IN=512, OUT=256, H=W=32, L=32).

Sharding: data-parallel over batch (2 items per core, 8 cores). Sync-BN:
BN1's batch statistics depend only on the input x, so they are computed
exactly on the host and folded into per-channel scale/shift vectors -
no device work and no collective needed. BN2's statistics depend on conv1's
device-computed output, so they use the cross-core AllReduce (one collective).

Design notes (v2):
  - gamma/beta algebra: gamma = A + (W_g @ wdn) @ attnT, so the host folds
    W_g @ wdn per batch item ([L=32, C] matrices) and the device contracts
    over L=32 instead of TD=256; the ctx matmuls disappear entirely
  - conv1/conv2 in fp8-e4m3 DoubleRow (2 fp8 MACs per PE cell per cycle):
    exact 3-term split W*A ~= W8*A8 + (W8*Alo + Wlo*A8), dropping only the
    ~(2^-4)^2 Wlo*Alo term. Main term: one DR instr per k-tile pair.
    Cross terms: one DR instr per k-tile carrying both products via the
    pair slots (lhsT=(W8,Wlo), rhs=(Alo,A8)). 25% fewer PE cycles than bf16
    at bf16-class accuracy. Activation split written by the relu sink (ACT,
    fp8 out) + one DVE residual pass per sink.
  - weights scaled by a power of 2 before the fp8 split so the residual
    terms clear e4m3's 2^-9 subnormal floor (tiny 0.02-scale weights would
    otherwise hit an absolute error floor); descale folded into the PSUM
    drains (ACT scale / DVE scalar operand)
  - img1/img2 matmuls single-fp8 DoubleRow (attention path tolerates it)
  - transposed softmax with per-partition norms; mask folded into sim matmul;
    word norms folded on host; DVE fast-rsqrt for image norms
  - conv1 = 4 subpixel 2x2 convs (host-folded weights); conv2 = 9-tap 3x3;
    both consume zero-border padded SBUF tiles (A8+Alo slots)
  - conv1 output kept in SBUF; bf16 copy for BN2-affine, batched fp8 copy
    for the img2 matmul
  - engine balance: squares on Pool, pointwise chains split DVE/ACT,
    stats-squares on ACT (accum), so PE (the bottleneck) stays fed
  - deep software pipelining: attention split into img-phase (A1) and
    sim-phase (A2) emitted two chunks apart so ACT/Pool drains never stall
    the in-order PE queue; 8-chunk attention lookahead in stage 2; all
    conv1 finishes before the sync-BN AllReduce, whose wait is covered by
    attention prefetch + the (input-only-dependent) shortcut matmuls
"""
import sys
sys.path.insert(0, "/opt/trn_rl_repo")

import contextlib
import numpy as np
import concourse.bass as bass
import concourse.bacc as bacc
import concourse.mybir as mybir
import concourse.tile as tile

F32 = mybir.dt.float32
BF16 = mybir.dt.bfloat16
F8 = mybir.dt.float8e4
I32 = mybir.dt.int32
AX = mybir.AxisListType
OP = mybir.AluOpType
ACT = mybir.ActivationFunctionType
DR = mybir.MatmulPerfMode.DoubleRow

B, IN, OUT, GD, TD, H, W, L = 16, 512, 256, 256, 256, 32, 32, 32
COND = GD + TD
EPS_BN = 1e-5
N_CORES = 8
B_LOC = B // N_CORES
P1 = H * W                    # 1024
P2 = 4 * P1                   # 4096
CHUNK = 512
KT1 = IN // 128               # 4
KT2 = OUT // 128              # 2
MT = TD // 128                # 2
NCH1 = P1 // CHUNK            # 2
NCH2 = P2 // CHUNK            # 8
NSL = CHUNK // 128            # 4 pixel slices per chunk
MASK_NEG = -500.0
RSQRT_C = 0x5F3759DF
GBW = 2 * IN + 2 * OUT        # per-item wgbw columns: [g1 IN | b1 IN | g2 OUT | b2 OUT]

# packed f32 vector-param layout (columns of the "vecs" input)
_VOFF = {}
_off = 0
for _nm, _kt in (("bsc", KT2), ("bn2w", KT2), ("bn2b", KT2),
                 ("s1", KT1), ("t1", KT1),
                 ("A1", KT1 * B_LOC), ("B1", KT1 * B_LOC),
                 ("A2", KT2 * B_LOC), ("B2", KT2 * B_LOC),
                 ("ics", 2)):
    _VOFF[_nm] = _off
    _off += _kt
VCOLS = _off


def build_program(num_devices=N_CORES, use_collectives=True):
    nc = bacc.Bacc("TRN2", target_bir_lowering=False, debug=False,
                   num_devices=num_devices)

    def din(name, shape, dt=BF16):
        return nc.dram_tensor(name, list(shape), dt, kind="ExternalInput")

    x_d = din("x", (B_LOC, IN, P1))
    x8_d = din("x8", (B_LOC, IN, P1), F8)
    # packed small bf16 constants: [wdn 128 | ident 128 | wdnT 512 | maskb 64]
    cbf_d = din("cbf", (128, 832))
    wimg1_d = din("w_img1T", (IN, TD), F8)
    wimg2_d = din("w_img2T", (OUT, TD), F8)
    wgbw_d = din("wgbw", (L, B_LOC * GBW))
    # host pre-permuted to SBUF layout: [p, s, k, q, t, o] / [p, s, k, t, o]
    w1s8_d = din("w1s8", (128, 2 * KT1 * 16 * OUT), F8)
    w2t8_d = din("w2t8", (128, 2 * KT2 * 9 * OUT), F8)
    wsc_d = din("w_scT", (IN, OUT))
    vecs_d = din("vecs", (128, VCOLS), F32)

    out_d = nc.dram_tensor("out", [B_LOC, OUT, P2], F32, kind="ExternalOutput")

    with tile.TileContext(nc) as tc:
        st = contextlib.ExitStack()
        cpool = st.enter_context(tc.tile_pool(name="cpool", bufs=1))
        scr = st.enter_context(tc.tile_pool(name="scr", bufs=2))
        psum = st.enter_context(tc.tile_pool(name="psum", bufs=1, space="PSUM"))
        dram = st.enter_context(tc.tile_pool(name="dram", bufs=1, space="DRAM"))
        s1x = contextlib.ExitStack()
        ph1x = s1x.enter_context(tc.tile_pool(name="ph1x", bufs=1))
        s1 = contextlib.ExitStack()
        ph1 = s1.enter_context(tc.tile_pool(name="ph1", bufs=1))
        ph1r = s1.enter_context(tc.tile_pool(name="ph1r", bufs=2))

        # ---------------- constants + batched loads (need-ordered) ----------
        ones_col = cpool.tile([128, 1], BF16, name="ones_col")
        nc.vector.memset(ones_col[:], 1.0)
        ones_pix = cpool.tile([1, 128], BF16, name="ones_pix")
        nc.vector.memset(ones_pix[:], 1.0)

        # first-need loads split fine so the first img1 matmul starts early
        wimg1_t = ph1.tile([128, KT1 * TD], F8, name="wimg1")
        wimg1v = wimg1_t[:].rearrange("p (k c) -> p k c", k=KT1)
        x8_sb = [ph1.tile([128, KT1 * P1], F8, name=f"x8_{b}") for b in range(B_LOC)]
        x_sb = [ph1x.tile([128, KT1 * P1], BF16, name=f"x_{b}") for b in range(B_LOC)]

        def x_ap(b, k, n):
            return x_sb[b][:, k * P1 + n * CHUNK:k * P1 + (n + 1) * CHUNK]

        def x8v(b):
            return x8_sb[b][:].rearrange("p (k n) -> p k n", k=KT1)

        def dma_x(b, n, ks=slice(0, KT1)):
            nc.sync.dma_start(
                x_sb[b][:].rearrange("p (k n) -> p k n", k=KT1)[:, ks, n * CHUNK:(n + 1) * CHUNK],
                x_d.ap()[b].rearrange("(k p) n -> p k n", p=128)[:, ks, n * CHUNK:(n + 1) * CHUNK])

        def dma_x8(b, n, ks=slice(0, KT1)):
            nc.sync.dma_start(
                x8_sb[b][:].rearrange("p (k n) -> p k n", k=KT1)[:, ks, n * CHUNK:(n + 1) * CHUNK],
                x8_d.ap()[b].rearrange("(k p) n -> p k n", p=128)[:, ks, n * CHUNK:(n + 1) * CHUNK])

        def dma_wimg1(ks):
            nc.sync.dma_start(
                wimg1_t[:].rearrange("p (k c) -> p k c", k=KT1)[:, ks],
                wimg1_d.ap().rearrange("(k p) c -> p k c", p=128)[:, ks])

        dma_wimg1(slice(0, 2))
        dma_x8(0, 0, slice(0, 2))
        dma_wimg1(slice(2, KT1))
        dma_x8(0, 0, slice(2, KT1))

        cbf = cpool.tile([128, 832], BF16, name="cbf")
        nc.sync.dma_start(cbf[:], cbf_d.ap())
        wdn_sb = [[cbf[:, (b * MT + m) * L:(b * MT + m + 1) * L]
                   for m in range(MT)] for b in range(B_LOC)]
        ident = cbf[:, 128:256]
        maskb_sb = [cbf[0:1, 768 + b * L:768 + (b + 1) * L] for b in range(B_LOC)]

        dma_x8(0, 1)
        dma_x(0, 0)

        wsc_t = ph1x.tile([128, KT1 * OUT], BF16, name="wsc")
        nc.sync.dma_start(wsc_t[:].rearrange("p (k c) -> p k c", k=KT1),
                          wsc_d.ap().rearrange("(k p) c -> p k c", p=128))
        wsc = [wsc_t[:, k * OUT:(k + 1) * OUT] for k in range(KT1)]

        vecs = cpool.tile([128, VCOLS], F32, name="vecs")
        nc.sync.dma_start(vecs[:], vecs_d.ap())

        def vcol(nm, k):
            o = _VOFF[nm] + k
            return vecs[:, o:o + 1]

        def vgrp(nm, kt):
            o = _VOFF[nm]
            return vecs[:, o:o + kt]

        wgbw = cpool.tile([L, B_LOC * GBW], BF16, name="wgbw")
        nc.sync.dma_start(wgbw[:], wgbw_d.ap())

        def wgbw_ap(b, stage, gb, m):
            # stage 1: [g1 | b1] at 0 / IN; stage 2: [g2 | b2] at 2*IN / 2*IN+OUT
            o = b * GBW + (0 if stage == 1 else 2 * IN) + \
                gb * (IN if stage == 1 else OUT) + m * 128
            return wgbw[:, o:o + 128]

        dma_x(0, 1)
        dma_x8(1, 0)
        dma_x8(1, 1)
        dma_x(1, 0)
        dma_x(1, 1)

        # conv1 fp8 weights, layout [s(hi,lo)][k][q][tap][o]
        w1t = ph1.tile([128, 2 * KT1 * 16 * OUT], F8, name="w1s8")
        nc.sync.dma_start(w1t[:], w1s8_d.ap())
        w1tv = w1t[:].rearrange("p (s k q t o) -> p s k q t o", s=2, k=KT1, q=4, t=4)

        # conv2 fp8 weights, layout [s][k][tap][o]
        w2t = cpool.tile([128, 2 * KT2 * 9 * OUT], F8, name="w2t8")
        nc.sync.dma_start(w2t[:], w2t8_d.ap())
        w2tv = w2t[:].rearrange("p (s k t o) -> p s k t o", s=2, k=KT2, t=9)

        wimg2_t = cpool.tile([128, KT2 * TD], F8, name="wimg2")
        nc.sync.dma_start(wimg2_t[:].rearrange("p (k c) -> p k c", k=KT2),
                          wimg2_d.ap().rearrange("(k p) c -> p k c", p=128))
        wimg2v = wimg2_t[:].rearrange("p (k c) -> p k c", k=KT2)

        # ---------------- fast rsqrt helper (DVE only) ----------------
        def fast_rsqrt(dst_ap, src_ap, cols, tag, iters=2):
            ti = scr.tile([128, cols], I32, name=f"rsq_i_{tag}", tag=f"rsq_i_{cols}", bufs=2)
            tf = scr.tile([128, cols], F32, name=f"rsq_f_{tag}", tag=f"rsq_f_{cols}", bufs=2)
            nc.vector.tensor_scalar(ti[:], src_ap.bitcast(I32), 1, None,
                                    op0=OP.logical_shift_right)
            nc.vector.tensor_scalar(ti[:], ti[:], -1, None, op0=OP.bitwise_xor)
            nc.vector.tensor_scalar(ti[:], ti[:], RSQRT_C + 1, None, op0=OP.add)
            y = ti[:].bitcast(F32)
            for it in range(iters):
                nc.vector.tensor_tensor(tf[:], y, y, OP.mult)
                nc.vector.tensor_tensor(tf[:], tf[:], src_ap, OP.mult)
                nc.vector.tensor_scalar(tf[:], tf[:], -0.5, 1.5, op0=OP.mult, op1=OP.add)
                if it < iters - 1:
                    nc.vector.tensor_tensor(y, y, tf[:], OP.mult)
                else:
                    nc.vector.tensor_tensor(dst_ap, y, tf[:], OP.mult)

        A1 = vgrp("A1", KT1 * B_LOC)
        B1 = vgrp("B1", KT1 * B_LOC)
        A2 = vgrp("A2", KT2 * B_LOC)
        B2 = vgrp("B2", KT2 * B_LOC)

        # ---------------- BN2 stats state + persistent outputs ----------
        sum2P = cpool.tile([128, KT2 * B_LOC * 8], F32, name="sum2P")
        ss2P = cpool.tile([128, KT2 * B_LOC * 8], F32, name="ss2P")
        sc_sb = [[cpool.tile([128, P1], BF16, name=f"sc_{b}_{m}") for m in range(KT2)]
                 for b in range(B_LOC)]
        out1_sb = [[cpool.tile([128, P2], BF16, name=f"o1_{b}_{m}") for m in range(KT2)]
                   for b in range(B_LOC)]
        out18_sb = [cpool.tile([128, KT2 * P2], F8, name=f"o18_{b}")
                    for b in range(B_LOC)]

        def out18v(b):
            return out18_sb[b][:].rearrange("p (k n) -> p k n", k=KT2)

        # ================= generic attention chunk =================
        # phase A1: img matmuls -> imgc (ACT drain) + squares (Pool).
        # phase A2 (emitted one slot later so A1's ACT/Pool chain is done and
        # never stalls the in-order PE queue): sim+norm matmuls -> SBUF spill.
        def attnA1(b, n, img_mm, name, extra=None):
            imgc = scr.tile([128, MT * CHUNK], BF16, name=f"imgc_{name}", tag="imgc", bufs=3)
            for m in range(MT):
                pim = psum.tile([128, CHUNK], F32, name=f"pim_{name}_{m}", tag="mm2", bufs=2)
                img_mm(pim, m)
                nc.scalar.copy(imgc[:, m * CHUNK:(m + 1) * CHUNK], pim[:])
            sq = scr.tile([128, MT * CHUNK], BF16, name=f"sq_{name}", tag="sq", bufs=3)
            nc.gpsimd.tensor_tensor(sq[:], imgc[:], imgc[:], OP.mult)
            if extra is not None:
                extra()
            return (b, n, imgc, sq, name)

        def attnA2(state):
            b, n, imgc, sq, name = state
            ps = psum.tile([128, NSL * L + NSL], F32, name=f"ps_{name}", tag="small", bufs=1)
            for s in range(NSL):
                for m in range(MT):
                    nc.tensor.matmul(ps[:, s * L:(s + 1) * L],
                                     imgc[:, m * CHUNK + s * 128:m * CHUNK + (s + 1) * 128],
                                     wdn_sb[b][m], start=(m == 0), stop=False)
                nc.tensor.matmul(ps[:, s * L:(s + 1) * L], ones_pix[:], maskb_sb[b],
                                 start=False, stop=True)
                for m in range(MT):
                    nc.tensor.matmul(ps[:, NSL * L + s:NSL * L + s + 1],
                                     sq[:, m * CHUNK + s * 128:m * CHUNK + (s + 1) * 128],
                                     ones_col[:], start=(m == 0), stop=(m == MT - 1))
            simr = scr.tile([128, NSL * L + NSL], F32, name=f"simr_{name}",
                            tag="simr", bufs=8)
            nc.scalar.copy(simr[:], ps[:])
            return simr

        def attnB(b, n, ps, name):
            invn = scr.tile([128, NSL], F32, name=f"invn_{name}", tag="invn", bufs=3)
            fast_rsqrt(invn[:], ps[:, NSL * L:NSL * L + NSL], NSL, name, iters=1)
            e = scr.tile([128, NSL * L], BF16, name=f"e_{name}", tag="e", bufs=3)
            den = scr.tile([128, 2 * NSL], F32, name=f"den_{name}", tag="den", bufs=3)
            for s in range(NSL):
                nc.scalar.activation(e[:, s * L:(s + 1) * L], ps[:, s * L:(s + 1) * L],
                                     ACT.Exp, bias=0.0, scale=invn[:, s:s + 1])
                nc.vector.tensor_reduce(den[:, s:s + 1], e[:, s * L:(s + 1) * L], AX.X, OP.add)
            nc.vector.reciprocal(den[:, NSL:2 * NSL], den[:, 0:NSL])
            ptr = psum.tile([32, NSL * 128], BF16, name=f"ptr_{name}", tag="tr", bufs=1)
            for s in range(NSL):
                nc.vector.tensor_scalar_mul(e[:, s * L:(s + 1) * L], e[:, s * L:(s + 1) * L],
                                            den[:, NSL + s:NSL + s + 1])
                nc.tensor.matmul(ptr[:, s * 128:(s + 1) * 128], e[:, s * L:(s + 1) * L],
                                 ident, is_transpose=True, start=True, stop=True)
            attnT = scr.tile([32, CHUNK], BF16, name=f"attnT_{name}", tag="attnT", bufs=7)
            nc.vector.tensor_copy(attnT[:], ptr[:])
            return attnT

        # gamma/beta matmuls (L=32 contraction) + bn-affine + relu into the A8
        # pad view + one DVE residual pass into the Alo pad view
        def gb_affine(name, b, attnT, stage, mt_out, Av, Bv, sv, tv,
                      bnx_src, sink_view, lo_view, bnx_act=False):
            for m in range(mt_out):
                pg = psum.tile([128, CHUNK], F32, name=f"pg_{name}_{m}", tag="mm", bufs=2)
                nc.tensor.matmul(pg[:], wgbw_ap(b, stage, 0, m), attnT[:],
                                 start=True, stop=True)
                pb = psum.tile([128, CHUNK], F32, name=f"pbb_{name}_{m}", tag="mm", bufs=2)
                nc.tensor.matmul(pb[:], wgbw_ap(b, stage, 1, m), attnT[:],
                                 start=True, stop=True)
                bnx = scr.tile([128, CHUNK], BF16, name=f"bnx_{name}_{m}",
                               tag="bnx", bufs=4)
                if bnx_act:
                    nc.scalar.activation(bnx[:], bnx_src(m), ACT.Identity,
                                         bias=tv(m), scale=sv(m))
                else:
                    nc.vector.tensor_scalar(bnx[:], bnx_src(m), sv(m), tv(m),
                                            op0=OP.mult, op1=OP.add)
                bB = scr.tile([128, CHUNK], BF16, name=f"bB_{name}_{m}",
                              tag="bB", bufs=4)
                if m % 2 == 0:
                    nc.scalar.activation(bB[:], pb[:], ACT.Identity,
                                         bias=Bv[:, m * B_LOC + b:m * B_LOC + b + 1], scale=1.0)
                else:
                    nc.vector.tensor_scalar(bB[:], pb[:],
                                            Bv[:, m * B_LOC + b:m * B_LOC + b + 1],
                                            None, op0=OP.add)
                t1 = scr.tile([128, CHUNK], BF16, name=f"t1_{name}_{m}",
                              tag="t1", bufs=4)
                nc.vector.scalar_tensor_tensor(
                    t1[:], pg[:], Av[:, m * B_LOC + b:m * B_LOC + b + 1],
                    bnx[:], OP.add, OP.mult)
                pre = scr.tile([128, CHUNK], BF16, name=f"pre_{name}_{m}",
                               tag="pre", bufs=4)
                nc.vector.tensor_tensor(pre[:], t1[:], bB[:], OP.add)
                pre_v = pre[:].rearrange("p (r c) -> p r c", r=16)
                nc.scalar.activation(sink_view(m), pre_v, ACT.Relu)
                # Alo = relu(pre) - A8  (captures the fp8 quantization residual)
                nc.vector.scalar_tensor_tensor(lo_view(m), pre_v, 0.0,
                                               sink_view(m), OP.max, OP.subtract)

        def border_memset(v, pw):
            # v: [p, s, k, r, c] padded view; zero borders of every (s, k) slot
            nc.gpsimd.memset(v[:, :, :, 0:1, :], 0.0)
            nc.gpsimd.memset(v[:, :, :, pw - 1:pw, :], 0.0)
            nc.gpsimd.memset(v[:, :, :, :, 0:1], 0.0)
            nc.gpsimd.memset(v[:, :, :, :, pw - 1:pw], 0.0)

        # ================= stage 1 (BN1 folded on host) + conv1 =============
        PW1 = 34
        pads1_all = []
        pads1v = []
        for b in range(B_LOC):
            t = ph1r.tile([128, 2 * KT1 * PW1 * PW1], F8, name=f"pad1_{b}",
                          tag="pad1", bufs=2)
            pads1_all.append(t)
            v = t[:].rearrange("p (s k r c) -> p s k r c", s=2, k=KT1, r=PW1)
            pads1v.append(v)
            border_memset(v, PW1)

        chunks1 = [(b, n) for b in range(B_LOC) for n in range(NCH1)]

        def s1_A(i):
            b, n = chunks1[i]
            name = f"s1_{b}_{n}"

            def img_mm(pim, m):
                for kp in range(0, KT1, 2):
                    nc.tensor.matmul(
                        pim[:], wimg1v[:, kp:kp + 2, m * 128:(m + 1) * 128],
                        x8v(b)[:, kp:kp + 2, n * CHUNK:(n + 1) * CHUNK],
                        start=(kp == 0), stop=(kp == KT1 - 2), perf_mode=DR)

            return attnA1(b, n, img_mm, name)

        def sc_chunk(b, n):
            x_aps = [x_ap(b, k, n) for k in range(KT1)]
            for m in range(KT2):
                psc = psum.tile([128, CHUNK], F32, name=f"psc_{b}_{n}_{m}", tag="mm", bufs=2)
                for k in range(KT1):
                    nc.tensor.matmul(psc[:], wsc[k][:, m * 128:(m + 1) * 128],
                                     x_aps[k], start=(k == 0), stop=(k == KT1 - 1))
                nc.scalar.activation(
                    sc_sb[b][m][:, n * CHUNK:(n + 1) * CHUNK], psc[:],
                    ACT.Identity, bias=vcol("bsc", m), scale=1.0)

        def s1_B(i, ps):
            b, n = chunks1[i]
            name = f"s1_{b}_{n}"
            attnT = attnB(b, n, ps, name)
            v = pads1v[b]
            gb_affine(
                name, b, attnT, 1, KT1, A1, B1,
                lambda m: vcol("s1", m), lambda m: vcol("t1", m),
                lambda m: x_ap(b, m, n),
                lambda m: v[:, 1, m, 1 + 16 * n:1 + 16 * (n + 1), 1:33],
                lambda m: v[:, 0, m, 1 + 16 * n:1 + 16 * (n + 1), 1:33])

        def emit_c1(b, q, m, n):
            v = pads1v[b]
            a_, b2_ = q // 2, q % 2
            roff = [0, 1] if a_ == 0 else [1, 2]
            coff = [0, 1] if b2_ == 0 else [1, 2]
            pc = psum.tile([128, CHUNK], F32, name=f"pc1_{b}_{q}_{m}_{n}",
                           tag="conv", bufs=2)
            mm = []   # (lhsT, rhs) DoubleRow ops
            for ti in range(4):
                si, tj = ti // 2, ti % 2
                r0 = 16 * n + roff[si]
                c0 = coff[tj]
                o0 = m * 128
                # main term: W8 over k-tile pairs x A8
                for kp in range(0, KT1, 2):
                    mm.append((w1tv[:, 0, kp:kp + 2, q, ti, o0:o0 + 128],
                               v[:, 1, kp:kp + 2, r0:r0 + 16, c0:c0 + 32]))
                # cross terms: (W8, Wlo) x (Alo, A8) per k-tile
                for k in range(KT1):
                    mm.append((w1tv[:, :, k, q, ti, o0:o0 + 128],
                               v[:, :, k, r0:r0 + 16, c0:c0 + 32]))
            for i, (lh, rh) in enumerate(mm):
                nc.tensor.matmul(pc[:], lh, rh, start=(i == 0),
                                 stop=(i == len(mm) - 1), perf_mode=DR)
            ci = (m * B_LOC + b) * 8 + q * NCH1 + n
            dst = out1_sb[b][m][:, q * P1 + n * CHUNK:q * P1 + (n + 1) * CHUNK]
            nc.scalar.activation(dst, pc[:], ACT.Copy, bias=0.0,
                                 scale=vcol("ics", 0),
                                 accum_out=sum2P[:, ci:ci + 1])
            thr = scr.tile([128, CHUNK], BF16, name=f"thr2_{b}_{q}_{m}_{n}",
                           tag="sq_throw", bufs=1)
            nc.scalar.activation(thr[:], dst, ACT.Square,
                                 accum_out=ss2P[:, ci:ci + 1])


        blocks1 = [(q, m, n) for q in range(4) for m in range(KT2)
                   for n in range(NCH1)]

        # emission: all img phases first (dense PE), then sims, then the
        # softmax/gb phases with conv1(b0) woven in
        pend1 = [s1_A(0), s1_A(1)]
        pA2 = {0: attnA2(pend1[0])}
        pend1.append(s1_A(2))
        pA2[1] = attnA2(pend1[1])
        s1_B(0, pA2.pop(0))
        pend1.append(s1_A(3))
        pA2[2] = attnA2(pend1[2])
        s1_B(1, pA2.pop(1))
        for q, m, n in blocks1[:4]:
            emit_c1(0, q, m, n)
        pA2[3] = attnA2(pend1[3])
        s1_B(2, pA2.pop(2))
        for q, m, n in blocks1[4:8]:
            emit_c1(0, q, m, n)
        s1_B(3, pA2.pop(3))
        for q, m, n in blocks1[8:]:
            emit_c1(0, q, m, n)
        for m in range(KT2):
            nc.vector.tensor_copy(out18v(0)[:, m, :], out1_sb[0][m][:])

        order2 = [0, 2, 4, 6, 1, 3, 5, 7]
        chunks2 = [(b, n) for b in range(B_LOC) for n in order2]

        def s2_A(j):
            b, n = chunks2[j]

            def img_mm(pim, m):
                nc.tensor.matmul(
                    pim[:], wimg2v[:, 0:2, m * 128:(m + 1) * 128],
                    out18v(b)[:, 0:2, n * CHUNK:(n + 1) * CHUNK],
                    start=True, stop=True, perf_mode=DR)

            return attnA1(b, n, img_mm, f"s2_{b}_{n}")

        def s2_B(j):
            b, n = chunks2[j]
            return attnB(b, n, ps_l.pop(j), f"s2_{b}_{n}")

        # conv1(b1) + early stage-2 A chunks interleaved; ALL conv1 blocks
        # finish before the BN2 stat reduce so the AllReduce launches early
        ps_l, att_l, a1_l = {}, {}, {}
        for q, m, n in blocks1[:3]:
            emit_c1(1, q, m, n)
        a1_l[0] = s2_A(0)
        for q, m, n in blocks1[3:5]:
            emit_c1(1, q, m, n)
        a1_l[1] = s2_A(1)
        ps_l[0] = attnA2(a1_l.pop(0))
        for q, m, n in blocks1[5:7]:
            emit_c1(1, q, m, n)
        a1_l[2] = s2_A(2)
        ps_l[1] = attnA2(a1_l.pop(1))
        att_l[0] = s2_B(0)
        for q, m, n in blocks1[7:9]:
            emit_c1(1, q, m, n)
        a1_l[3] = s2_A(3)
        ps_l[2] = attnA2(a1_l.pop(2))
        att_l[1] = s2_B(1)
        for q, m, n in blocks1[9:11]:
            emit_c1(1, q, m, n)
        a1_l[4] = s2_A(4)
        ps_l[3] = attnA2(a1_l.pop(3))
        att_l[2] = s2_B(2)
        for q, m, n in blocks1[11:13]:
            emit_c1(1, q, m, n)
        a1_l[5] = s2_A(5)
        ps_l[4] = attnA2(a1_l.pop(4))
        att_l[3] = s2_B(3)
        for q, m, n in blocks1[13:16]:
            emit_c1(1, q, m, n)
        for m in range(KT2):
            nc.vector.tensor_copy(out18v(1)[:, m, :], out1_sb[1][m][:])

        # ---- BN2 AllReduce (all conv1 stats are now emitted) ----
        ar2_in = dram.tile([KT2, 128, 2], F32, name="ar2_in")
        ar2_out = dram.tile([KT2, 128, 2], F32, name="ar2_out",
                            addr_space="Shared" if use_collectives else "Local")
        st2 = cpool.tile([128, 2 * KT2], F32, name="st2")
        for k in range(KT2):
            nc.vector.tensor_reduce(st2[:, 2 * k:2 * k + 1],
                                    sum2P[:, 16 * k:16 * (k + 1)], AX.X, OP.add)
            nc.vector.tensor_reduce(st2[:, 2 * k + 1:2 * k + 2],
                                    ss2P[:, 16 * k:16 * (k + 1)], AX.X, OP.add)
            nc.sync.dma_start(ar2_in[k], st2[:, 2 * k:2 * k + 2])
        if use_collectives:
            nc.gpsimd.collective_compute(
                "AllReduce", OP.add, replica_groups=[list(range(num_devices))],
                ins=[ar2_in.opt()], outs=[ar2_out.opt()])
        else:
            nc.sync.dma_start(ar2_out[:], ar2_in[:])
        B_STATS = B if use_collectives else B_LOC

        # independent attention work around the AllReduce wait; bn_post sits
        # early in the DVE queue (before the attnB chains) so the first
        # s2_GB drains don't gridlock the matmul psum rotation
        a1_l[6] = s2_A(6)
        ps_l[5] = attnA2(a1_l.pop(5))
        s1.close()

        # bn_post: scale/shift from the AllReduced stats (DVE fast-rsqrt)
        g2 = cpool.tile([128, 2 * KT2], F32, name="g2")
        s2v = cpool.tile([128, KT2], F32, name="s2v")
        t2v = cpool.tile([128, KT2], F32, name="t2v")
        mean2 = cpool.tile([128, KT2], F32, name="mean2")
        var2 = cpool.tile([128, KT2], F32, name="var2")
        istd2 = cpool.tile([128, KT2], F32, name="istd2")
        n_total2 = B_STATS * P2
        nc.sync.dma_start(g2[:].rearrange("p (k c) -> p k c", k=KT2),
                          ar2_out[:, :, :].rearrange("k p c -> p k c"))
        for k in range(KT2):
            nc.vector.tensor_scalar_mul(mean2[:, k:k + 1], g2[:, 2 * k:2 * k + 1],
                                        1.0 / n_total2)
            nc.vector.scalar_tensor_tensor(var2[:, k:k + 1], mean2[:, k:k + 1], 0.0,
                                           mean2[:, k:k + 1], OP.add, OP.mult)
            nc.vector.scalar_tensor_tensor(var2[:, k:k + 1], g2[:, 2 * k + 1:2 * k + 2],
                                           1.0 / n_total2, var2[:, k:k + 1],
                                           OP.mult, OP.subtract)
        nc.vector.tensor_scalar_add(var2[:], var2[:], float(EPS_BN))
        fast_rsqrt(istd2[:], var2[:], KT2, "bn2")
        nc.vector.tensor_tensor(s2v[:], istd2[:], vgrp("bn2w", KT2), OP.mult)
        nc.vector.tensor_tensor(t2v[:], mean2[:], s2v[:], OP.mult)
        nc.vector.tensor_tensor(t2v[:], vgrp("bn2b", KT2), t2v[:], OP.subtract)

        # ================= stage 2 + conv2 =================
        s2 = contextlib.ExitStack()
        ph2 = s2.enter_context(tc.tile_pool(name="ph2", bufs=2))
        PW2 = 66

        pads2_all = []
        pads2v = []
        for b in range(B_LOC):
            t = ph2.tile([128, 2 * KT2 * PW2 * PW2], F8, name=f"pad2_{b}",
                         tag="pad2", bufs=2)
            pads2_all.append(t)
            v = t[:].rearrange("p (s k r c) -> p s k r c", s=2, k=KT2, r=PW2)
            pads2v.append(v)
            border_memset(v, PW2)

        a1_l[7] = s2_A(7)
        att_l[4] = s2_B(4)

        def s2_GB(j, attnT):
            b, n = chunks2[j]
            name = f"s2_{b}_{n}"
            v = pads2v[b]
            qq, hh = n // 2, n % 2
            aq, bq = qq // 2, qq % 2
            r0 = 1 + aq + 32 * hh
            c0 = 1 + bq
            gb_affine(
                name, b, attnT, 2, KT2, A2, B2,
                lambda m: s2v[:, m:m + 1], lambda m: t2v[:, m:m + 1],
                lambda m: out1_sb[b][m][:, n * CHUNK:(n + 1) * CHUNK],
                lambda m: v[:, 1, m, r0:r0 + 32:2, c0:c0 + 64:2],
                lambda m: v[:, 0, m, r0:r0 + 32:2, c0:c0 + 64:2])

        def conv2_mm(b, m, n, rows, h):
            # DoubleRow op list for rows [8n + rows*h, +rows)
            v = pads2v[b]
            o0 = m * 128
            mm = []
            for t in range(9):
                ku, kv = t // 3, t % 3
                r0 = 8 * n + rows * h + ku
                mm.append((w2tv[:, 0, 0:2, t, o0:o0 + 128],
                           v[:, 1, 0:2, r0:r0 + rows, kv:kv + 64]))
                for k in range(KT2):
                    mm.append((w2tv[:, :, k, t, o0:o0 + 128],
                               v[:, :, k, r0:r0 + rows, kv:kv + 64]))
            return mm

        def conv2_drain(b, m, n, pc, cols, h):
            rows = cols // 64
            ri = rows // 2
            fin = scr.tile([128, cols], F32, name=f"fin_{b}_{m}_{n}_{h}",
                           tag="fin", bufs=4)
            scv = sc_sb[b][m][:].rearrange("p (i j) -> p i j", i=32)[
                :, 4 * n + ri * h:4 * n + ri * h + ri, :]
            scv4 = scv.unsqueeze(3).to_broadcast((128, ri, 32, 2))
            for a_ in (0, 1):
                nc.vector.scalar_tensor_tensor(
                    fin[:].rearrange("p (i a j c) -> p i a j c", i=ri, a=2, j=32)[:, :, a_],
                    pc[:].rearrange("p (i a j c) -> p i a j c", i=ri, a=2, j=32)[:, :, a_],
                    vcol("ics", 1), scv4, OP.mult, OP.add)
            nc.sync.dma_start(
                out_d.ap()[b, m * 128:(m + 1) * 128,
                           n * CHUNK + h * cols:n * CHUNK + (h + 1) * cols], fin[:])

        def conv2_block(b, m, n):
            pc = psum.tile([128, CHUNK], F32, name=f"pc2_{b}_{m}_{n}",
                           tag="conv", bufs=2)
            mm = conv2_mm(b, m, n, 8, 0)
            for i, (lh, rh) in enumerate(mm):
                nc.tensor.matmul(pc[:], lh, rh, start=(i == 0),
                                 stop=(i == len(mm) - 1), perf_mode=DR)
            conv2_drain(b, m, n, pc, CHUNK, 0)

        # conv2 readiness within each b's [0,2,4,6,1,3,5,7] stream:
        # blocks 0-2 after pos 3, block 3 after pos 5, blocks 4-7 after pos 7
        sched = {}
        for b in range(B_LOC):
            base = NCH2 * b
            for j, nb in ((3, 0), (4, 1), (5, 2), (6, 3),
                          (8, 4), (9, 5), (10, 6), (11, 7)):
                sched.setdefault(base + j, []).append((b, nb))

        NC2 = len(chunks2)
        conv_q = []
        for g in range(NC2):
            if g < 4:
                sc_chunk(g // 2, g % 2)
            if g + 8 < NC2:
                a1_l[g + 8] = s2_A(g + 8)
            if g + 6 < NC2 and (g + 6) in a1_l:
                ps_l[g + 6] = attnA2(a1_l.pop(g + 6))
            if g + 5 < NC2:
                att_l[g + 5] = s2_B(g + 5)
            # only blocks whose sinks completed in EARLIER iterations may run
            # ahead of this chunk's gamma/beta+sink chain
            for bb, m, nb in conv_q[:1]:
                conv2_block(bb, m, nb)
            conv_q = conv_q[1:]
            s2_GB(g, att_l.pop(g))
            for bb, nb in sched.get(g, []):
                conv_q += [(bb, m, nb) for m in range(KT2)]
            for bb, m, nb in conv_q[:2]:
                conv2_block(bb, m, nb)
            conv_q = conv_q[2:]

        def conv2_block_part(b, m, n, h, nh):
            rows = 8 // nh
            cols = rows * 64
            pc = psum.tile([128, cols], F32, name=f"pc2h_{b}_{m}_{n}_{h}",
                           tag="conv", bufs=2)
            mm = conv2_mm(b, m, n, rows, h)
            for i, (lh, rh) in enumerate(mm):
                nc.tensor.matmul(pc[:], lh, rh, start=(i == 0),
                                 stop=(i == len(mm) - 1), perf_mode=DR)
            conv2_drain(b, m, n, pc, cols, h)

        conv_q += [(bb, m, nb) for g in range(NC2, NC2 + NCH2)
                   for bb, nb in sched.get(g, []) for m in range(KT2)]
        for bb, m, nb in conv_q[:-3]:
            conv2_block(bb, m, nb)
        for bb, m, nb in conv_q[-3:-1]:
            for h in range(2):
                conv2_block_part(bb, m, nb, h, 2)
        bb, m, nb = conv_q[-1]
        for h in range(4):
            conv2_block_part(bb, m, nb, h, 4)
        s2.close()
        s1x.close()
        st.close()

    nc.compile()
    return nc


# ---------------------------------------------------------------------------
# host side
# ---------------------------------------------------------------------------
_cached = {}


def _to_bf16(a):
    import ml_dtypes
    return np.ascontiguousarray(np.asarray(a, np.float32).astype(ml_dtypes.bfloat16))


def _to_f8(a):
    import ml_dtypes
    return np.ascontiguousarray(np.asarray(a, np.float32).astype(ml_dtypes.float8_e4m3))


def _split_f8(a):
    """Scale by a power of 2 so the fp8 residual clears the e4m3 subnormal
    floor (2^-9), split into hi+lo, return (hi, lo, 1/scale)."""
    import ml_dtypes
    a = np.asarray(a, np.float32)
    s = float(2.0 ** np.floor(np.log2(120.0 / max(1e-30, np.abs(a).max()))))
    s = min(max(s, 1.0), 2.0 ** 14)
    hi = (a * s).astype(ml_dtypes.float8_e4m3)
    lo = (a * s - hi.astype(np.float32)).astype(ml_dtypes.float8_e4m3)
    return hi, lo, 1.0 / s


def _pack_vecs(inputs, gc_core, ics):
    v = np.zeros((128, VCOLS), np.float32)
    v[:, _VOFF["ics"]] = ics[0]
    v[:, _VOFF["ics"] + 1] = ics[1]

    def put(nm, a):
        a = np.asarray(a, np.float32)
        kt = a.size // 128
        v[:, _VOFF[nm]:_VOFF[nm] + kt] = a.reshape(kt, 128).T

    put("bsc", inputs["b_sc"])
    put("bn2w", inputs["bn2_w"])
    put("bn2b", inputs["bn2_b"])
    # BN1 stats depend only on the input x: fold exactly on host (sync-BN
    # over the full batch, matching the reference's training-mode BN)
    x = np.asarray(inputs["x"], np.float64)
    mean = x.mean(axis=(0, 2, 3))
    var = x.var(axis=(0, 2, 3))
    s1 = np.asarray(inputs["bn1_w"], np.float64) / np.sqrt(var + EPS_BN)
    t1 = np.asarray(inputs["bn1_b"], np.float64) - mean * s1
    put("s1", s1)
    put("t1", t1)
    # per-batch-item gamma/beta from the global-cond half
    for nm, wkey, bkey in (("A1", "w_g1", "b_g1"), ("B1", "w_b1", "b_b1"),
                           ("A2", "w_g2", "b_g2"), ("B2", "w_b2", "b_b2")):
        wgc = np.asarray(inputs[wkey], np.float32)[:, :GD]
        bias = np.asarray(inputs[bkey], np.float32)
        ab = wgc @ gc_core.T + bias[:, None]          # [cout, B_LOC]
        kt = ab.shape[0] // 128
        ab = ab.reshape(kt, 128, B_LOC).transpose(1, 0, 2).reshape(128, kt * B_LOC)
        v[:, _VOFF[nm]:_VOFF[nm] + kt * B_LOC] = ab
    return np.ascontiguousarray(v)


def _prep_weights(inputs):
    wb = {}
    wb["w_img1T"] = _to_f8(inputs["w_img1"].T)
    wb["w_img2T"] = _to_f8(inputs["w_img2"].T)
    wb["w_scT"] = _to_bf16(inputs["w_sc"][:, :, 0, 0].T)

    wc1 = np.asarray(inputs["w_c1"], np.float32)
    rows = {0: [[0], [1, 2]], 1: [[0, 1], [2]]}
    w1sub = np.zeros((4, 4, IN, OUT), np.float32)
    for a in (0, 1):
        for b2 in (0, 1):
            q = a * 2 + b2
            for si in (0, 1):
                for tj in (0, 1):
                    acc = np.zeros((OUT, IN), np.float32)
                    for ku in rows[a][si]:
                        for kv in rows[b2][tj]:
                            acc += wc1[:, :, ku, kv]
                    w1sub[q, si * 2 + tj] = acc.T
    hi, lo, ics1 = _split_f8(w1sub)
    st = np.stack([hi, lo], axis=0)               # [s, q, t, IN, OUT]
    wb["w1s8"] = np.ascontiguousarray(
        st.reshape(2, 4, 4, KT1, 128, OUT).transpose(4, 0, 3, 1, 2, 5)
        .reshape(128, -1))                        # [p, s, k, q, t, o]
    wc2 = np.asarray(inputs["w_c2"], np.float32)
    w2taps = np.zeros((9, OUT, OUT), np.float32)
    for t in range(9):
        w2taps[t] = wc2[:, :, t // 3, t % 3].T
    hi, lo, ics2 = _split_f8(w2taps)
    st = np.stack([hi, lo], axis=0)               # [s, t, OUT, OUT]
    wb["w2t8"] = np.ascontiguousarray(
        st.reshape(2, 9, KT2, 128, OUT).transpose(3, 0, 2, 1, 4)
        .reshape(128, -1))                        # [p, s, k, t, o]
    return wb, (ics1, ics2)


def make_in_maps(inputs):
    w, ics = _prep_weights(inputs)
    x = np.asarray(inputs["x"], np.float32).reshape(B, IN, P1)
    gc = np.asarray(inputs["global_cond"], np.float32)
    words = np.asarray(inputs["words_embs"], np.float32)
    norm = np.sqrt((words * words).sum(axis=1, keepdims=True))
    wdn = words / np.maximum(norm, 1e-12)
    maskb = np.where(np.asarray(inputs["mask"]), np.float32(MASK_NEG), np.float32(0.0))
    wg1 = np.asarray(inputs["w_g1"], np.float32)[:, GD:]
    wb1 = np.asarray(inputs["w_b1"], np.float32)[:, GD:]
    wg2 = np.asarray(inputs["w_g2"], np.float32)[:, GD:]
    wb2 = np.asarray(inputs["w_b2"], np.float32)[:, GD:]
    in_maps = []
    for c in range(N_CORES):
        sl = slice(c * B_LOC, (c + 1) * B_LOC)
        m = dict(w)
        m["x"] = _to_bf16(x[sl])
        m["x8"] = _to_f8(x[sl])
        m["vecs"] = _pack_vecs(inputs, gc[sl], ics)
        # packed small constants [wdn | ident | wdnT(unused) | maskb]
        cbf = np.zeros((128, 832), np.float32)
        wdn_c = wdn[sl]                                   # [B_LOC, TD, L]
        for b in range(B_LOC):
            for mt in range(MT):
                cbf[:, (b * MT + mt) * L:(b * MT + mt + 1) * L] = \
                    wdn_c[b, mt * 128:(mt + 1) * 128, :]
        cbf[:, 128:256] = np.eye(128, dtype=np.float32)
        for b in range(B_LOC):
            cbf[0:1, 768 + b * L:768 + (b + 1) * L] = maskb[sl][b]
        m["cbf"] = _to_bf16(cbf)
        # host-folded W_g @ wdn per item: [L, IN|IN|OUT|OUT] per b
        gbw = np.zeros((L, B_LOC * GBW), np.float32)
        for b in range(B_LOC):
            o = b * GBW
            gbw[:, o:o + IN] = wdn_c[b].T @ wg1.T
            gbw[:, o + IN:o + 2 * IN] = wdn_c[b].T @ wb1.T
            gbw[:, o + 2 * IN:o + 2 * IN + OUT] = wdn_c[b].T @ wg2.T
            gbw[:, o + 2 * IN + OUT:o + GBW] = wdn_c[b].T @ wb2.T
        m["wgbw"] = _to_bf16(gbw)
        in_maps.append(m)
    return in_maps


def kernel(**inputs):
    from concourse.bass_utils import run_bass_kernel_spmd
    if "nc" not in _cached:
        _cached["nc"] = build_program()
    nc = _cached["nc"]
    in_maps = make_in_maps(inputs)
    res = run_bass_kernel_spmd(nc, in_maps, core_ids=list(range(N_CORES)))
    out = np.empty((B, OUT, 2 * H, 2 * W), np.float32)
    for c in range(N_CORES):
        out[c * B_LOC:(c + 1) * B_LOC] = res.results[c]["out"].reshape(B_LOC, OUT, 2 * H, 2 * W)
    return out
